# revision 4
# baseline (speedup 1.0000x reference)
"""PCGCv2-style point cloud encoder on 8 TRN2 NeuronCores (Bass/Tile).

Data-parallel over point-cloud partitions (per the sharding hint): output
rows of every level are sharded contiguously across the 8 cores. Sparse
convs run gather-first: for each 128-row output tile the K neighbor rows
are fetched from the replicated source table in DRAM via indirect DMAs,
transposed on the PE, and contracted against host-flattened weights with
PSUM accumulation. After each table is produced it is AllGathered so the
next layer's gathers can address the full row space (neighbor maps are
uniform-random, so halos are global and full replication is the right
exchange). Host does integer rulebook prep only (per-core neighbor offset
blocks, weight flattening/bias replication) plus level-0 input staging
(x_feat value per (node, tap) — standard sparse-conv rulebook staging for
a 1-channel input).
"""
import sys
sys.path.insert(0, '/opt/trn_rl_repo')
import numpy as np

import concourse.bass as bass
import concourse.mybir as mybir
import concourse.tile as tile
from concourse import bacc
from concourse.bass_utils import run_bass_kernel_spmd
from concourse.masks import make_identity

F32 = mybir.dt.float32
I32 = mybir.dt.int32
AF = mybir.ActivationFunctionType
ALU = mybir.AluOpType

NC_CORES = 8
P = 128


def _split_excess_waits(nc, maxw=1):
    """This walrus supports a single sync-wait per instruction; hoist the
    rest onto same-engine NoOps placed immediately before."""
    tot = 0
    for f in nc.m.functions:
        for bb in f.blocks:
            newlist = []
            changed = False
            for ins in bb.instructions:
                si = ins.sync_info
                if si is not None and len(si.on_wait) > maxw:
                    waits = list(si.on_wait)
                    extra, keep = waits[:-maxw], waits[-maxw:]
                    for j in range(0, len(extra), maxw):
                        nop = mybir.InstNoOp(name=f"I-wsplit-{tot}-{j}")
                        nop.engine = ins.engine
                        nop.sync_info = mybir.SyncInfo(on_wait=extra[j:j + maxw], on_update=[])
                        newlist.append(nop)
                    ins.sync_info = mybir.SyncInfo(on_wait=keep, on_update=list(si.on_update))
                    tot += 1
                    changed = True
                newlist.append(ins)
            if changed:
                bb.instructions = newlist
    return tot


def _cdiv(a, b):
    return -(-a // b)


def _flat_w(w):
    K, Cin, Cout = w.shape
    return np.ascontiguousarray(w.reshape(K * Cin, Cout)).astype(np.float32)


def _joint_w(w01, w11):
    # joint gathered rows are k-major blocks of [a1(C4) | b1(C4)]
    K, C4, C2 = w01.shape
    W = np.zeros((K * 2 * C4, C2 + C4), np.float32)
    for k in range(K):
        W[k * 2 * C4: k * 2 * C4 + C4, :C2] = w01[k]
        W[k * 2 * C4 + C4: k * 2 * C4 + 2 * C4, C2:] = w11[k]
    return W


def _rep(b):
    return np.tile(np.asarray(b, np.float32)[None, :], (P, 1))


def _pad_rows(a, tiles, dtype):
    rows, K = a.shape
    out = np.zeros((tiles * P, K), dtype)
    out[:rows] = a
    return out


def build_net(cfg):
    N = cfg["N"]
    sh = [n // NC_CORES for n in N]
    tl = [_cdiv(s, P) for s in sh]
    CH = [16, 32, 64, 32]

    nc = bacc.Bacc(None, target_bir_lowering=False)
    inp = {}

    def _in(name, shape, dtype=F32):
        t = nc.dram_tensor(name, list(shape), dtype, kind="ExternalInput")
        inp[name] = t
        return t

    g0 = _in("g0", (tl[0] * P, 27))
    offs = {
        "d0": _in("offs_d0", (tl[1] * P, 8), I32),
        "l1": _in("offs_l1", (tl[1] * P, 27), I32),
        "d1": _in("offs_d1", (tl[2] * P, 8), I32),
        "l2": _in("offs_l2", (tl[2] * P, 27), I32),
        "d2": _in("offs_d2", (tl[3] * P, 8), I32),
        "l3": _in("offs_l3", (tl[3] * P, 27), I32),
    }
    noise = _in("noise_sh", (tl[3] * P, 8))
    wt = {k: _in("w_" + k, v) for k, v in cfg["weights"].items()}

    def shared(name, rows, cols):
        return nc.dram_tensor(name, [rows, cols], F32, addr_space="Shared")

    T0f = shared("T0f", N[0], CH[0])
    F1f = [shared(f"F1_{i}", N[1], CH[1]) for i in range(4)]
    AB1f = [shared(f"AB1_{i}", N[1], CH[1] // 2) for i in range(3)]
    out0f = shared("out0f", N[1], CH[1])
    T1df = shared("T1df", N[1], CH[1])
    F2f = [shared(f"F2_{i}", N[2], CH[2]) for i in range(4)]
    AB2f = [shared(f"AB2_{i}", N[2], CH[2] // 2) for i in range(3)]
    out1f = shared("out1f", N[2], CH[2])
    T2df = shared("T2df", N[2], CH[2])
    F3f = [shared(f"F3_{i}", N[3], CH[3]) for i in range(4)]
    AB3f = [shared(f"AB3_{i}", N[3], CH[3] // 2) for i in range(3)]
    out2f = shared("out2f", N[3], CH[3])

    out0_o = nc.dram_tensor("out0_sh", [sh[1], CH[1]], F32, kind="ExternalOutput")
    out1_o = nc.dram_tensor("out1_sh", [sh[2], CH[2]], F32, kind="ExternalOutput")
    y_o = nc.dram_tensor("y_sh", [sh[3], 8], F32, kind="ExternalOutput")
    comp_o = nc.dram_tensor("comp_sh", [sh[3], 8], F32, kind="ExternalOutput")

    db = cfg["db"]

    with tile.TileContext(nc) as tc:
        with (
            tc.tile_pool(name="wp", bufs=1) as wpool,
            tc.tile_pool(name="dp", bufs=1, space="DRAM") as dpool,
            tc.tile_pool(name="sb", bufs=3) as pool,
            tc.tile_pool(name="gtq", bufs=3) as gtp,
            tc.tile_pool(name="ps", bufs=3, space="PSUM") as psp,
            tc.tile_pool(name="pst", bufs=4, space="PSUM") as pstp,
        ):
            ident = wpool.tile([P, P], F32, tag="ident")
            make_identity(nc, ident[:])

            W = {}
            for k, t in wt.items():
                r, c = t.shape
                w_tile = wpool.tile([P, _cdiv(r, P) * c], F32, tag="w_" + k)
                for b in range(_cdiv(r, P)):
                    rr = min(P, r - b * P)
                    nc.sync.dma_start(w_tile[:rr, b * c:(b + 1) * c], t[b * P:b * P + rr, :])
                W[k] = (w_tile, r, c)

            def wblk(name, b):
                w_tile, r, c = W[name]
                rr = min(P, r - b * P)
                return w_tile[:rr, b * c:(b + 1) * c]

            def transpose_blk(src_ap, rows, cols):
                tp = pstp.tile([P, P], F32, tag="tp")
                nc.tensor.transpose(tp[:cols, :rows], src_ap, ident[:rows, :rows])
                ts = pool.tile([P, P], F32, tag="ts")
                nc.vector.tensor_copy(ts[:cols, :rows], tp[:cols, :rows])
                return ts

            def gmm(gt_tile, Pr, KC, wname, Cout):
                nblk = _cdiv(KC, P)
                acc = psp.tile([P, Cout], F32, tag="acc")
                for b in range(nblk):
                    w = min(P, KC - b * P)
                    ts = transpose_blk(gt_tile[:Pr, b * P:b * P + w], Pr, w)
                    nc.tensor.matmul(acc[:Pr, :Cout], ts[:w, :Pr], wblk(wname, b),
                                     start=(b == 0), stop=(b == nblk - 1))
                return acc

            def gather(offs_t, src_full, Pr, K, Cin):
                gt = gtp.tile([P, K * Cin], F32, tag="gt")
                for k in range(K):
                    nc.gpsimd.indirect_dma_start(
                        out=gt[:Pr, k * Cin:(k + 1) * Cin],
                        out_offset=None,
                        in_=src_full[:],
                        in_offset=bass.IndirectOffsetOnAxis(ap=offs_t[:Pr, k:k + 1], axis=0),
                    )
                return gt

            def load_offs(which, t, K):
                ot = pool.tile([P, K], I32, tag="offs")
                nc.sync.dma_start(ot[:], offs[which][t * P:(t + 1) * P, :])
                return ot

            def add_bias(acc, Pr, bname, relu=False):
                _, _, c = W[bname]
                o = pool.tile([P, c], F32, tag="ab_" + str(c))
                nc.vector.tensor_tensor(out=o[:Pr, :], in0=acc[:Pr, :c],
                                        in1=wblk(bname, 0)[:Pr, :c], op=ALU.add)
                if relu:
                    nc.scalar.activation(o[:Pr, :], o[:Pr, :], AF.Relu)
                return o

            def mask_apply(f_rm, Pr, C, pre):
                fT = transpose_blk(f_rm[:Pr, :C], Pr, C)
                h_ps = psp.tile([P, C // 2], F32, tag="acc")
                nc.tensor.matmul(h_ps[:Pr, :], fT[:C, :Pr], wblk(f"p{pre}_w1", 0),
                                 start=True, stop=True)
                h = add_bias(h_ps, Pr, f"p{pre}_b1", relu=True)
                tmp = pool.tile([P, C // 2], F32, tag="mtmp")
                nc.vector.tensor_tensor(out=tmp[:Pr, :], in0=h[:Pr, :],
                                        in1=wblk(f"p{pre}_dw", 0)[:Pr, :], op=ALU.mult)
                d = pool.tile([P, 1], F32, tag="md")
                nc.vector.tensor_reduce(d[:Pr, :], tmp[:Pr, :],
                                        axis=mybir.AxisListType.X, op=ALU.add)
                m = pool.tile([P, 1], F32, tag="mm")
                nc.scalar.activation(m[:Pr, :], d[:Pr, :], AF.Sigmoid,
                                     bias=wblk(f"p{pre}_db", 0)[:Pr, :1])
                fm = pool.tile([P, C], F32, tag="mfm")
                nc.vector.tensor_scalar(out=fm[:Pr, :], in0=f_rm[:Pr, :C],
                                        scalar1=m[:Pr, :1], scalar2=None, op0=ALU.mult)
                o = pool.tile([P, C], F32, tag="mout")
                nc.vector.tensor_tensor(out=o[:Pr, :], in0=f_rm[:Pr, :C],
                                        in1=fm[:Pr, :], op=ALU.add)
                return o

            def allgather(shard_tile, rows_sh, full):
                nc.gpsimd.collective_compute(
                    "AllGather", ALU.bypass,
                    replica_groups=[list(range(NC_CORES))],
                    ins=[shard_tile[:rows_sh, :]],
                    outs=[full[:]],
                )

            def rows_in(t, level_rows):
                return min(P, level_rows - t * P)

            # ---------------- Level 0 ----------------
            T0sh = dpool.tile([tl[0] * P, CH[0]], F32, tag="T0sh")
            for t in range(tl[0]):
                Pr = rows_in(t, sh[0])
                g0t = pool.tile([P, 27], F32, tag="g0t")
                nc.sync.dma_start(g0t[:Pr, :], g0[t * P:t * P + Pr, :])
                gT = transpose_blk(g0t[:Pr, :27], Pr, 27)
                f_ps = psp.tile([P, CH[0]], F32, tag="acc")
                nc.tensor.matmul(f_ps[:Pr, :], gT[:27, :Pr], wblk("conv0", 0),
                                 start=True, stop=True)
                f0 = add_bias(f_ps, Pr, "conv0_b")
                fo = mask_apply(f0, Pr, CH[0], 3)
                nc.scalar.activation(fo[:Pr, :], fo[:Pr, :], AF.Relu)
                nc.sync.dma_start(T0sh[t * P:t * P + Pr, :], fo[:Pr, :])
            allgather(T0sh, sh[0], T0f)

            def down(src_f, dst_f, offs_key, ntile, rows_sh, K, Cin, Cout, wname):
                dsh = dpool.tile([ntile * P, Cout], F32, tag="dsh_" + wname)
                for t in range(ntile):
                    Pr = rows_in(t, rows_sh)
                    ot = load_offs(offs_key, t, K)
                    gt = gather(ot, src_f, Pr, K, Cin)
                    acc = gmm(gt, Pr, K * Cin, wname, Cout)
                    o = add_bias(acc, Pr, wname + "_b", relu=True)
                    nc.sync.dma_start(dsh[t * P:t * P + Pr, :], o[:Pr, :])
                allgather(dsh, rows_sh, dst_f)
                return dsh

            def irn(Fin_f, Fin_sh, Fout_f, ABf, offs_key, ntile, rows_sh, C, pre):
                C4, C2 = C // 4, C // 2
                ABsh = dpool.tile([ntile * P, C2], F32, tag="ABsh_" + pre)
                for t in range(ntile):
                    Pr = rows_in(t, rows_sh)
                    ot = load_offs(offs_key, t, 27)
                    gt = gather(ot, Fin_f, Pr, 27, C)
                    acc = gmm(gt, Pr, 27 * C, f"{pre}_w00", C4)
                    a1 = add_bias(acc, Pr, f"{pre}_b00", relu=True)
                    frm = pool.tile([P, C], F32, tag="frm")
                    nc.sync.dma_start(frm[:Pr, :], Fin_sh[t * P:t * P + Pr, :])
                    fT = transpose_blk(frm[:Pr, :C], Pr, C)
                    b_ps = psp.tile([P, C4], F32, tag="acc")
                    nc.tensor.matmul(b_ps[:Pr, :], fT[:C, :Pr], wblk(f"{pre}_w10", 0),
                                     start=True, stop=True)
                    b1 = add_bias(b_ps, Pr, f"{pre}_b10", relu=True)
                    comb = pool.tile([P, C2], F32, tag="comb")
                    nc.vector.tensor_copy(comb[:Pr, :C4], a1[:Pr, :C4])
                    nc.vector.tensor_copy(comb[:Pr, C4:C2], b1[:Pr, :C4])
                    nc.sync.dma_start(ABsh[t * P:t * P + Pr, :], comb[:Pr, :])
                allgather(ABsh, rows_sh, ABf)
                Fosh = dpool.tile([ntile * P, C], F32, tag="Fosh_" + pre)
                for t in range(ntile):
                    Pr = rows_in(t, rows_sh)
                    ot = load_offs(offs_key, t, 27)
                    gt = gather(ot, ABf, Pr, 27, C2)
                    acc = gmm(gt, Pr, 27 * C2, f"{pre}_wj", C2 + C4)
                    ab = add_bias(acc, Pr, f"{pre}_bj")
                    btmp = pool.tile([P, C4], F32, tag="btmp")
                    nc.scalar.activation(btmp[:Pr, :], ab[:Pr, C2:C2 + C4], AF.Relu)
                    bT = transpose_blk(btmp[:Pr, :C4], Pr, C4)
                    bf_ps = psp.tile([P, C2], F32, tag="acc")
                    nc.tensor.matmul(bf_ps[:Pr, :], bT[:C4, :Pr], wblk(f"{pre}_w12", 0),
                                     start=True, stop=True)
                    bfin = add_bias(bf_ps, Pr, f"{pre}_b12")
                    frm = pool.tile([P, C], F32, tag="frm")
                    nc.sync.dma_start(frm[:Pr, :], Fin_sh[t * P:t * P + Pr, :])
                    o = pool.tile([P, C], F32, tag="irnout")
                    nc.vector.tensor_tensor(out=o[:Pr, :C2], in0=bfin[:Pr, :C2],
                                            in1=frm[:Pr, :C2], op=ALU.add)
                    nc.vector.tensor_tensor(out=o[:Pr, C2:], in0=ab[:Pr, :C2],
                                            in1=frm[:Pr, C2:], op=ALU.add)
                    nc.sync.dma_start(Fosh[t * P:t * P + Pr, :], o[:Pr, :])
                allgather(Fosh, rows_sh, Fout_f)
                return Fosh

            def mask_level(Fin_sh, outf, out_o, ntile, rows_sh, C, pre):
                osh = dpool.tile([ntile * P, C], F32, tag=f"osh{pre}")
                for t in range(ntile):
                    Pr = rows_in(t, rows_sh)
                    frm = pool.tile([P, C], F32, tag="frm")
                    nc.sync.dma_start(frm[:Pr, :], Fin_sh[t * P:t * P + Pr, :])
                    o = mask_apply(frm, Pr, C, pre)
                    nc.sync.dma_start(osh[t * P:t * P + Pr, :], o[:Pr, :])
                    if out_o is not None:
                        nc.sync.dma_start(out_o[t * P:t * P + Pr, :], o[:Pr, :])
                allgather(osh, rows_sh, outf)
                return osh

            def conv_relu(src_f, dst_f, offs_key, ntile, rows_sh, C, wname):
                csh = dpool.tile([ntile * P, C], F32, tag="csh_" + wname)
                for t in range(ntile):
                    Pr = rows_in(t, rows_sh)
                    ot = load_offs(offs_key, t, 27)
                    gt = gather(ot, src_f, Pr, 27, C)
                    acc = gmm(gt, Pr, 27 * C, wname, C)
                    o = add_bias(acc, Pr, wname + "_b", relu=True)
                    nc.sync.dma_start(csh[t * P:t * P + Pr, :], o[:Pr, :])
                allgather(csh, rows_sh, dst_f)

            # ---------------- L1 ----------------
            fsh = down(T0f, F1f[0], "d0", tl[1], sh[1], 8, CH[0], CH[1], "down0")
            for i in range(3):
                fsh = irn(F1f[i], fsh, F1f[i + 1], AB1f[i], "l1", tl[1], sh[1], CH[1], f"b0_{i}")
            mask_level(fsh, out0f, out0_o, tl[1], sh[1], CH[1], 4)
            conv_relu(out0f, T1df, "l1", tl[1], sh[1], CH[1], "conv1")
            # ---------------- L2 ----------------
            fsh = down(T1df, F2f[0], "d1", tl[2], sh[2], 8, CH[1], CH[2], "down1")
            for i in range(3):
                fsh = irn(F2f[i], fsh, F2f[i + 1], AB2f[i], "l2", tl[2], sh[2], CH[2], f"b1_{i}")
            mask_level(fsh, out1f, out1_o, tl[2], sh[2], CH[2], 5)
            conv_relu(out1f, T2df, "l2", tl[2], sh[2], CH[2], "conv2")
            # ---------------- L3 ----------------
            fsh = down(T2df, F3f[0], "d2", tl[3], sh[3], 8, CH[2], CH[3], "down2")
            for i in range(3):
                fsh = irn(F3f[i], fsh, F3f[i + 1], AB3f[i], "l3", tl[3], sh[3], CH[3], f"b2_{i}")
            mask_level(fsh, out2f, None, tl[3], sh[3], CH[3], 6)
            # conv3 + compressed
            for t in range(tl[3]):
                Pr = rows_in(t, sh[3])
                ot = load_offs("l3", t, 27)
                gt = gather(ot, out2f, Pr, 27, CH[3])
                acc = gmm(gt, Pr, 27 * CH[3], "conv3", 8)
                yv = add_bias(acc, Pr, "conv3_b")
                nc.sync.dma_start(y_o[t * P:t * P + Pr, :], yv[:Pr, :])
                nz = pool.tile([P, 8], F32, tag="nz")
                nc.sync.dma_start(nz[:Pr, :], noise[t * P:t * P + Pr, :])
                cmp_t = pool.tile([P, 8], F32, tag="cmp")
                nc.vector.tensor_tensor(out=cmp_t[:Pr, :], in0=yv[:Pr, :],
                                        in1=nz[:Pr, :], op=ALU.add)
                nc.vector.tensor_scalar_add(cmp_t[:Pr, :], cmp_t[:Pr, :], -0.5)
                nc.sync.dma_start(comp_o[t * P:t * P + Pr, :], cmp_t[:Pr, :])

    nc.compile()
    _split_excess_waits(nc)
    return nc


def prepare(inputs, cfg):
    N = cfg["N"]
    sh = [n // NC_CORES for n in N]
    tl = [_cdiv(s, P) for s in sh]
    p = inputs["params"]
    x = np.asarray(inputs["x_feat"], np.float32)[:, 0]

    weights = {}
    weights["conv0"] = np.asarray(p["conv0_w"], np.float32)[:, 0, :]
    weights["conv0_b"] = _rep(p["conv0_b"])
    for blk, pre in [("block0", "b0"), ("block1", "b1"), ("block2", "b2")]:
        for i, q in enumerate(p[blk]):
            weights[f"{pre}_{i}_w00"] = _flat_w(np.asarray(q["w00"], np.float32))
            weights[f"{pre}_{i}_b00"] = _rep(q["b00"])
            weights[f"{pre}_{i}_w10"] = np.asarray(q["w10"], np.float32)
            weights[f"{pre}_{i}_b10"] = _rep(q["b10"])
            weights[f"{pre}_{i}_wj"] = _joint_w(np.asarray(q["w01"], np.float32),
                                                np.asarray(q["w11"], np.float32))
            weights[f"{pre}_{i}_bj"] = _rep(np.concatenate([np.asarray(q["b01"]),
                                                            np.asarray(q["b11"])]))
            weights[f"{pre}_{i}_w12"] = np.asarray(q["w12"], np.float32)
            weights[f"{pre}_{i}_b12"] = _rep(q["b12"])
    for name in ["conv1", "conv2", "conv3"]:
        weights[name] = _flat_w(np.asarray(p[name + "_w"], np.float32))
        weights[name + "_b"] = _rep(p[name + "_b"])
    for name in ["down0", "down1", "down2"]:
        weights[name] = _flat_w(np.asarray(p[name + "_w"], np.float32))
        weights[name + "_b"] = _rep(p[name + "_b"])
    db = {}
    for pre in [3, 4, 5, 6]:
        q = p[f"p{pre}"]
        weights[f"p{pre}_w1"] = np.asarray(q["w1"], np.float32)
        weights[f"p{pre}_b1"] = _rep(q["b1"])
        w2 = np.asarray(q["w2"], np.float32)
        b2 = np.asarray(q["b2"], np.float32)
        weights[f"p{pre}_dw"] = _rep(w2[:, 0] - w2[:, 1])
        db[pre] = float(b2[0] - b2[1])
        weights[f"p{pre}_db"] = np.full((P, 1), db[pre], np.float32)
    cfg["db"] = db
    cfg["weights"] = {k: v.shape for k, v in weights.items()}

    g0_full = x[np.asarray(inputs["nbr0_idx"])]
    maps = []
    for c in range(NC_CORES):
        m = {}
        m["g0"] = _pad_rows(g0_full[c * sh[0]:(c + 1) * sh[0]], tl[0], np.float32)
        m["offs_d0"] = _pad_rows(np.asarray(inputs["down0_idx"], np.int32)[c * sh[1]:(c + 1) * sh[1]], tl[1], np.int32)
        m["offs_l1"] = _pad_rows(np.asarray(inputs["nbr1_idx"], np.int32)[c * sh[1]:(c + 1) * sh[1]], tl[1], np.int32)
        m["offs_d1"] = _pad_rows(np.asarray(inputs["down1_idx"], np.int32)[c * sh[2]:(c + 1) * sh[2]], tl[2], np.int32)
        m["offs_l2"] = _pad_rows(np.asarray(inputs["nbr2_idx"], np.int32)[c * sh[2]:(c + 1) * sh[2]], tl[2], np.int32)
        m["offs_d2"] = _pad_rows(np.asarray(inputs["down2_idx"], np.int32)[c * sh[3]:(c + 1) * sh[3]], tl[3], np.int32)
        m["offs_l3"] = _pad_rows(np.asarray(inputs["nbr3_idx"], np.int32)[c * sh[3]:(c + 1) * sh[3]], tl[3], np.int32)
        m["noise_sh"] = _pad_rows(np.asarray(inputs["noise"], np.float32)[c * sh[3]:(c + 1) * sh[3]], tl[3], np.float32)
        for k, v in weights.items():
            m["w_" + k] = v
        maps.append(m)
    return maps


def kernel(x_feat, noise, params, nbr0_idx, down0_idx, nbr1_idx, down1_idx,
           nbr2_idx, down2_idx, nbr3_idx):
    inputs = dict(x_feat=x_feat, noise=noise, params=params, nbr0_idx=nbr0_idx,
                  down0_idx=down0_idx, nbr1_idx=nbr1_idx, down1_idx=down1_idx,
                  nbr2_idx=nbr2_idx, down2_idx=down2_idx, nbr3_idx=nbr3_idx)
    cfg = {"N": [int(np.asarray(nbr0_idx).shape[0]), int(np.asarray(nbr1_idx).shape[0]),
                 int(np.asarray(nbr2_idx).shape[0]), int(np.asarray(nbr3_idx).shape[0])]}
    maps = prepare(inputs, cfg)
    nc = build_net(cfg)
    res = run_bass_kernel_spmd(nc, maps, core_ids=list(range(NC_CORES)), trace=False)
    N = cfg["N"]
    sh = [n // NC_CORES for n in N]
    out0 = np.concatenate([np.asarray(res.results[c]["out0_sh"])[:sh[1]] for c in range(NC_CORES)])
    out1 = np.concatenate([np.asarray(res.results[c]["out1_sh"])[:sh[2]] for c in range(NC_CORES)])
    y = np.concatenate([np.asarray(res.results[c]["y_sh"])[:sh[3]] for c in range(NC_CORES)])
    comp = np.concatenate([np.asarray(res.results[c]["comp_sh"])[:sh[3]] for c in range(NC_CORES)])
    return (y, out1, out0, comp)


# revision 5
# speedup vs baseline: 1.4573x; 1.4573x over previous
"""PCGCv2-style point cloud encoder on 8 TRN2 NeuronCores (Bass/Tile).

Data-parallel over point-cloud partitions (per the sharding hint): output
rows of every level are sharded contiguously across the 8 cores. Sparse
convs run gather-first: for each 128-row output tile the K neighbor rows
are fetched from the replicated source table in DRAM via indirect DMAs,
transposed on the PE, and contracted against host-flattened weights with
PSUM accumulation. After each table is produced it is AllGathered so the
next layer's gathers can address the full row space (neighbor maps are
uniform-random, so halos are global and full replication is the right
exchange). Host does integer rulebook prep only (per-core neighbor offset
blocks, weight flattening/bias replication) plus level-0 input staging
(x_feat value per (node, tap) — standard sparse-conv rulebook staging for
a 1-channel input).
"""
import sys
sys.path.insert(0, '/opt/trn_rl_repo')
import numpy as np

import concourse.bass as bass
import concourse.mybir as mybir
import concourse.tile as tile
from concourse import bacc
from concourse.bass_utils import run_bass_kernel_spmd
from concourse.masks import make_identity

F32 = mybir.dt.float32
I32 = mybir.dt.int32
AF = mybir.ActivationFunctionType
ALU = mybir.AluOpType

NC_CORES = 8
P = 128


def _split_excess_waits(nc, maxw=1):
    """This walrus supports a single sync-wait per instruction; hoist the
    rest onto same-engine NoOps placed immediately before."""
    tot = 0
    for f in nc.m.functions:
        for bb in f.blocks:
            newlist = []
            changed = False
            for ins in bb.instructions:
                si = ins.sync_info
                if si is not None and len(si.on_wait) > maxw:
                    waits = list(si.on_wait)
                    extra, keep = waits[:-maxw], waits[-maxw:]
                    for j in range(0, len(extra), maxw):
                        nop = mybir.InstNoOp(name=f"I-wsplit-{tot}-{j}")
                        nop.engine = ins.engine
                        nop.sync_info = mybir.SyncInfo(on_wait=extra[j:j + maxw], on_update=[])
                        newlist.append(nop)
                    ins.sync_info = mybir.SyncInfo(on_wait=keep, on_update=list(si.on_update))
                    tot += 1
                    changed = True
                newlist.append(ins)
            if changed:
                bb.instructions = newlist
    return tot


def _cdiv(a, b):
    return -(-a // b)


def _flat_w(w):
    K, Cin, Cout = w.shape
    return np.ascontiguousarray(w.reshape(K * Cin, Cout)).astype(np.float32)


def _joint_w(w01, w11):
    # joint gathered rows are k-major blocks of [a1(C4) | b1(C4)]
    K, C4, C2 = w01.shape
    W = np.zeros((K * 2 * C4, C2 + C4), np.float32)
    for k in range(K):
        W[k * 2 * C4: k * 2 * C4 + C4, :C2] = w01[k]
        W[k * 2 * C4 + C4: k * 2 * C4 + 2 * C4, C2:] = w11[k]
    return W


def _rep(b):
    return np.tile(np.asarray(b, np.float32)[None, :], (P, 1))


def _pad_rows(a, tiles, dtype):
    rows, K = a.shape
    out = np.zeros((tiles * P, K), dtype)
    out[:rows] = a
    return out


def build_net(cfg):
    N = cfg["N"]
    sh = [n // NC_CORES for n in N]
    tl = [_cdiv(s, P) for s in sh]
    CH = [16, 32, 64, 32]

    nc = bacc.Bacc(None, target_bir_lowering=False)
    inp = {}

    def _in(name, shape, dtype=F32):
        t = nc.dram_tensor(name, list(shape), dtype, kind="ExternalInput")
        inp[name] = t
        return t

    g0 = _in("g0", (tl[0] * P, 27))
    offs = {
        "d0": _in("offs_d0", (tl[1] * P, 8), I32),
        "l1": _in("offs_l1", (tl[1] * P, 27), I32),
        "d1": _in("offs_d1", (tl[2] * P, 8), I32),
        "l2": _in("offs_l2", (tl[2] * P, 27), I32),
        "d2": _in("offs_d2", (tl[3] * P, 8), I32),
        "l3": _in("offs_l3", (tl[3] * P, 27), I32),
    }
    noise = _in("noise_sh", (tl[3] * P, 8))
    wt = {k: _in("w_" + k, v) for k, v in cfg["weights"].items()}

    def shared(name, rows, cols):
        return nc.dram_tensor(name, [rows, cols], F32, addr_space="Shared")

    T0f = shared("T0f", N[0], CH[0])
    F1f = [shared(f"F1_{i}", N[1], CH[1]) for i in range(4)]
    AB1f = [shared(f"AB1_{i}", N[1], CH[1] // 2) for i in range(3)]
    out0f = shared("out0f", N[1], CH[1])
    T1df = shared("T1df", N[1], CH[1])
    F2f = [shared(f"F2_{i}", N[2], CH[2]) for i in range(4)]
    AB2f = [shared(f"AB2_{i}", N[2], CH[2] // 2) for i in range(3)]
    out1f = shared("out1f", N[2], CH[2])
    T2df = shared("T2df", N[2], CH[2])
    F3f = [shared(f"F3_{i}", N[3], CH[3]) for i in range(4)]
    AB3f = [shared(f"AB3_{i}", N[3], CH[3] // 2) for i in range(3)]
    out2f = shared("out2f", N[3], CH[3])

    out0_o = nc.dram_tensor("out0_sh", [sh[1], CH[1]], F32, kind="ExternalOutput")
    out1_o = nc.dram_tensor("out1_sh", [sh[2], CH[2]], F32, kind="ExternalOutput")
    y_o = nc.dram_tensor("y_sh", [sh[3], 8], F32, kind="ExternalOutput")
    comp_o = nc.dram_tensor("comp_sh", [sh[3], 8], F32, kind="ExternalOutput")

    db = cfg["db"]

    with tile.TileContext(nc) as tc:
        with (
            tc.tile_pool(name="wp", bufs=1) as wpool,
            tc.tile_pool(name="dp", bufs=1, space="DRAM") as dpool,
            tc.tile_pool(name="sb", bufs=4) as pool,
            tc.tile_pool(name="gtq", bufs=6) as gtp,
            tc.tile_pool(name="ps", bufs=3, space="PSUM") as psp,
            tc.tile_pool(name="pst", bufs=4, space="PSUM") as pstp,
        ):
            ident = wpool.tile([P, P], F32, tag="ident")
            make_identity(nc, ident[:])

            W = {}
            for k, t in wt.items():
                r, c = t.shape
                w_tile = wpool.tile([P, _cdiv(r, P) * c], F32, tag="w_" + k)
                for b in range(_cdiv(r, P)):
                    rr = min(P, r - b * P)
                    nc.sync.dma_start(w_tile[:rr, b * c:(b + 1) * c], t[b * P:b * P + rr, :])
                W[k] = (w_tile, r, c)

            def wblk(name, b):
                w_tile, r, c = W[name]
                rr = min(P, r - b * P)
                return w_tile[:rr, b * c:(b + 1) * c]

            def transpose_blk(src_ap, rows, cols):
                tp = pstp.tile([P, P], F32, tag="tp")
                nc.tensor.transpose(tp[:cols, :rows], src_ap, ident[:rows, :rows])
                ts = pool.tile([P, P], F32, tag="ts")
                nc.vector.tensor_copy(ts[:cols, :rows], tp[:cols, :rows])
                return ts

            def gmm(gt_tile, Pr, KC, wname, Cout):
                nblk = _cdiv(KC, P)
                acc = psp.tile([P, Cout], F32, tag="acc")
                for b in range(nblk):
                    w = min(P, KC - b * P)
                    ts = transpose_blk(gt_tile[:Pr, b * P:b * P + w], Pr, w)
                    nc.tensor.matmul(acc[:Pr, :Cout], ts[:w, :Pr], wblk(wname, b),
                                     start=(b == 0), stop=(b == nblk - 1))
                return acc

            def gather(offs_t, src_full, Pr, K, Cin):
                gt = gtp.tile([P, K * Cin], F32, tag="gt")
                for k in range(K):
                    nc.gpsimd.indirect_dma_start(
                        out=gt[:Pr, k * Cin:(k + 1) * Cin],
                        out_offset=None,
                        in_=src_full[:],
                        in_offset=bass.IndirectOffsetOnAxis(ap=offs_t[:Pr, k:k + 1], axis=0),
                    )
                return gt

            def load_offs(which, t, K):
                ot = pool.tile([P, K], I32, tag="offs")
                nc.sync.dma_start(ot[:], offs[which][t * P:(t + 1) * P, :])
                return ot

            def add_bias(acc, Pr, bname, relu=False):
                _, _, c = W[bname]
                o = pool.tile([P, c], F32, tag="ab_" + str(c))
                nc.vector.tensor_tensor(out=o[:Pr, :], in0=acc[:Pr, :c],
                                        in1=wblk(bname, 0)[:Pr, :c], op=ALU.add)
                if relu:
                    nc.scalar.activation(o[:Pr, :], o[:Pr, :], AF.Relu)
                return o

            def mask_apply(f_rm, Pr, C, pre):
                fT = transpose_blk(f_rm[:Pr, :C], Pr, C)
                h_ps = psp.tile([P, C // 2], F32, tag="acc")
                nc.tensor.matmul(h_ps[:Pr, :], fT[:C, :Pr], wblk(f"p{pre}_w1", 0),
                                 start=True, stop=True)
                h = add_bias(h_ps, Pr, f"p{pre}_b1", relu=True)
                tmp = pool.tile([P, C // 2], F32, tag="mtmp")
                nc.vector.tensor_tensor(out=tmp[:Pr, :], in0=h[:Pr, :],
                                        in1=wblk(f"p{pre}_dw", 0)[:Pr, :], op=ALU.mult)
                d = pool.tile([P, 1], F32, tag="md")
                nc.vector.tensor_reduce(d[:Pr, :], tmp[:Pr, :],
                                        axis=mybir.AxisListType.X, op=ALU.add)
                m = pool.tile([P, 1], F32, tag="mm")
                nc.scalar.activation(m[:Pr, :], d[:Pr, :], AF.Sigmoid,
                                     bias=wblk(f"p{pre}_db", 0)[:Pr, :1])
                fm = pool.tile([P, C], F32, tag="mfm")
                nc.vector.tensor_scalar(out=fm[:Pr, :], in0=f_rm[:Pr, :C],
                                        scalar1=m[:Pr, :1], scalar2=None, op0=ALU.mult)
                o = pool.tile([P, C], F32, tag="mout")
                nc.vector.tensor_tensor(out=o[:Pr, :], in0=f_rm[:Pr, :C],
                                        in1=fm[:Pr, :], op=ALU.add)
                return o

            def allgather(shard_tile, rows_sh, full):
                nc.gpsimd.collective_compute(
                    "AllGather", ALU.bypass,
                    replica_groups=[list(range(NC_CORES))],
                    ins=[shard_tile[:rows_sh, :]],
                    outs=[full[:]],
                )

            def rows_in(t, level_rows):
                return min(P, level_rows - t * P)

            # ---------------- Level 0 ----------------
            T0sh = dpool.tile([tl[0] * P, CH[0]], F32, tag="T0sh")
            for t in range(tl[0]):
                Pr = rows_in(t, sh[0])
                g0t = pool.tile([P, 27], F32, tag="g0t")
                nc.sync.dma_start(g0t[:Pr, :], g0[t * P:t * P + Pr, :])
                gT = transpose_blk(g0t[:Pr, :27], Pr, 27)
                f_ps = psp.tile([P, CH[0]], F32, tag="acc")
                nc.tensor.matmul(f_ps[:Pr, :], gT[:27, :Pr], wblk("conv0", 0),
                                 start=True, stop=True)
                f0 = add_bias(f_ps, Pr, "conv0_b")
                fo = mask_apply(f0, Pr, CH[0], 3)
                nc.scalar.activation(fo[:Pr, :], fo[:Pr, :], AF.Relu)
                nc.sync.dma_start(T0sh[t * P:t * P + Pr, :], fo[:Pr, :])
            allgather(T0sh, sh[0], T0f)

            def down(src_f, dst_f, offs_key, ntile, rows_sh, K, Cin, Cout, wname):
                dsh = dpool.tile([ntile * P, Cout], F32, tag="dsh_" + wname)
                for t in range(ntile):
                    Pr = rows_in(t, rows_sh)
                    ot = load_offs(offs_key, t, K)
                    gt = gather(ot, src_f, Pr, K, Cin)
                    acc = gmm(gt, Pr, K * Cin, wname, Cout)
                    o = add_bias(acc, Pr, wname + "_b", relu=True)
                    nc.sync.dma_start(dsh[t * P:t * P + Pr, :], o[:Pr, :])
                allgather(dsh, rows_sh, dst_f)
                return dsh

            def irn(Fin_f, Fin_sh, Fout_f, ABf, offs_key, ntile, rows_sh, C, pre):
                C4, C2 = C // 4, C // 2
                ABsh = dpool.tile([ntile * P, C2], F32, tag="ABsh_" + pre)
                for t in range(ntile):
                    Pr = rows_in(t, rows_sh)
                    ot = load_offs(offs_key, t, 27)
                    gt = gather(ot, Fin_f, Pr, 27, C)
                    acc = gmm(gt, Pr, 27 * C, f"{pre}_w00", C4)
                    a1 = add_bias(acc, Pr, f"{pre}_b00", relu=True)
                    frm = pool.tile([P, C], F32, tag="frm")
                    nc.sync.dma_start(frm[:Pr, :], Fin_sh[t * P:t * P + Pr, :])
                    fT = transpose_blk(frm[:Pr, :C], Pr, C)
                    b_ps = psp.tile([P, C4], F32, tag="acc")
                    nc.tensor.matmul(b_ps[:Pr, :], fT[:C, :Pr], wblk(f"{pre}_w10", 0),
                                     start=True, stop=True)
                    b1 = add_bias(b_ps, Pr, f"{pre}_b10", relu=True)
                    comb = pool.tile([P, C2], F32, tag="comb")
                    nc.vector.tensor_copy(comb[:Pr, :C4], a1[:Pr, :C4])
                    nc.vector.tensor_copy(comb[:Pr, C4:C2], b1[:Pr, :C4])
                    nc.sync.dma_start(ABsh[t * P:t * P + Pr, :], comb[:Pr, :])
                allgather(ABsh, rows_sh, ABf)
                Fosh = dpool.tile([ntile * P, C], F32, tag="Fosh_" + pre)
                for t in range(ntile):
                    Pr = rows_in(t, rows_sh)
                    ot = load_offs(offs_key, t, 27)
                    gt = gather(ot, ABf, Pr, 27, C2)
                    acc = gmm(gt, Pr, 27 * C2, f"{pre}_wj", C2 + C4)
                    ab = add_bias(acc, Pr, f"{pre}_bj")
                    btmp = pool.tile([P, C4], F32, tag="btmp")
                    nc.scalar.activation(btmp[:Pr, :], ab[:Pr, C2:C2 + C4], AF.Relu)
                    bT = transpose_blk(btmp[:Pr, :C4], Pr, C4)
                    bf_ps = psp.tile([P, C2], F32, tag="acc")
                    nc.tensor.matmul(bf_ps[:Pr, :], bT[:C4, :Pr], wblk(f"{pre}_w12", 0),
                                     start=True, stop=True)
                    bfin = add_bias(bf_ps, Pr, f"{pre}_b12")
                    frm = pool.tile([P, C], F32, tag="frm")
                    nc.sync.dma_start(frm[:Pr, :], Fin_sh[t * P:t * P + Pr, :])
                    o = pool.tile([P, C], F32, tag="irnout")
                    nc.vector.tensor_tensor(out=o[:Pr, :C2], in0=bfin[:Pr, :C2],
                                            in1=frm[:Pr, :C2], op=ALU.add)
                    nc.vector.tensor_tensor(out=o[:Pr, C2:], in0=ab[:Pr, :C2],
                                            in1=frm[:Pr, C2:], op=ALU.add)
                    nc.sync.dma_start(Fosh[t * P:t * P + Pr, :], o[:Pr, :])
                allgather(Fosh, rows_sh, Fout_f)
                return Fosh

            def mask_level(Fin_sh, outf, out_o, ntile, rows_sh, C, pre):
                osh = dpool.tile([ntile * P, C], F32, tag=f"osh{pre}")
                for t in range(ntile):
                    Pr = rows_in(t, rows_sh)
                    frm = pool.tile([P, C], F32, tag="frm")
                    nc.sync.dma_start(frm[:Pr, :], Fin_sh[t * P:t * P + Pr, :])
                    o = mask_apply(frm, Pr, C, pre)
                    nc.sync.dma_start(osh[t * P:t * P + Pr, :], o[:Pr, :])
                    if out_o is not None:
                        nc.sync.dma_start(out_o[t * P:t * P + Pr, :], o[:Pr, :])
                allgather(osh, rows_sh, outf)
                return osh

            def conv_relu(src_f, dst_f, offs_key, ntile, rows_sh, C, wname):
                csh = dpool.tile([ntile * P, C], F32, tag="csh_" + wname)
                for t in range(ntile):
                    Pr = rows_in(t, rows_sh)
                    ot = load_offs(offs_key, t, 27)
                    gt = gather(ot, src_f, Pr, 27, C)
                    acc = gmm(gt, Pr, 27 * C, wname, C)
                    o = add_bias(acc, Pr, wname + "_b", relu=True)
                    nc.sync.dma_start(csh[t * P:t * P + Pr, :], o[:Pr, :])
                allgather(csh, rows_sh, dst_f)

            # ---------------- L1 ----------------
            fsh = down(T0f, F1f[0], "d0", tl[1], sh[1], 8, CH[0], CH[1], "down0")
            for i in range(3):
                fsh = irn(F1f[i], fsh, F1f[i + 1], AB1f[i], "l1", tl[1], sh[1], CH[1], f"b0_{i}")
            mask_level(fsh, out0f, out0_o, tl[1], sh[1], CH[1], 4)
            conv_relu(out0f, T1df, "l1", tl[1], sh[1], CH[1], "conv1")
            # ---------------- L2 ----------------
            fsh = down(T1df, F2f[0], "d1", tl[2], sh[2], 8, CH[1], CH[2], "down1")
            for i in range(3):
                fsh = irn(F2f[i], fsh, F2f[i + 1], AB2f[i], "l2", tl[2], sh[2], CH[2], f"b1_{i}")
            mask_level(fsh, out1f, out1_o, tl[2], sh[2], CH[2], 5)
            conv_relu(out1f, T2df, "l2", tl[2], sh[2], CH[2], "conv2")
            # ---------------- L3 ----------------
            fsh = down(T2df, F3f[0], "d2", tl[3], sh[3], 8, CH[2], CH[3], "down2")
            for i in range(3):
                fsh = irn(F3f[i], fsh, F3f[i + 1], AB3f[i], "l3", tl[3], sh[3], CH[3], f"b2_{i}")
            mask_level(fsh, out2f, None, tl[3], sh[3], CH[3], 6)
            # conv3 + compressed
            for t in range(tl[3]):
                Pr = rows_in(t, sh[3])
                ot = load_offs("l3", t, 27)
                gt = gather(ot, out2f, Pr, 27, CH[3])
                acc = gmm(gt, Pr, 27 * CH[3], "conv3", 8)
                yv = add_bias(acc, Pr, "conv3_b")
                nc.sync.dma_start(y_o[t * P:t * P + Pr, :], yv[:Pr, :])
                nz = pool.tile([P, 8], F32, tag="nz")
                nc.sync.dma_start(nz[:Pr, :], noise[t * P:t * P + Pr, :])
                cmp_t = pool.tile([P, 8], F32, tag="cmp")
                nc.vector.tensor_tensor(out=cmp_t[:Pr, :], in0=yv[:Pr, :],
                                        in1=nz[:Pr, :], op=ALU.add)
                nc.vector.tensor_scalar_add(cmp_t[:Pr, :], cmp_t[:Pr, :], -0.5)
                nc.sync.dma_start(comp_o[t * P:t * P + Pr, :], cmp_t[:Pr, :])

    nc.compile()
    _split_excess_waits(nc)
    return nc


def prepare(inputs, cfg):
    N = cfg["N"]
    sh = [n // NC_CORES for n in N]
    tl = [_cdiv(s, P) for s in sh]
    p = inputs["params"]
    x = np.asarray(inputs["x_feat"], np.float32)[:, 0]

    weights = {}
    weights["conv0"] = np.asarray(p["conv0_w"], np.float32)[:, 0, :]
    weights["conv0_b"] = _rep(p["conv0_b"])
    for blk, pre in [("block0", "b0"), ("block1", "b1"), ("block2", "b2")]:
        for i, q in enumerate(p[blk]):
            weights[f"{pre}_{i}_w00"] = _flat_w(np.asarray(q["w00"], np.float32))
            weights[f"{pre}_{i}_b00"] = _rep(q["b00"])
            weights[f"{pre}_{i}_w10"] = np.asarray(q["w10"], np.float32)
            weights[f"{pre}_{i}_b10"] = _rep(q["b10"])
            weights[f"{pre}_{i}_wj"] = _joint_w(np.asarray(q["w01"], np.float32),
                                                np.asarray(q["w11"], np.float32))
            weights[f"{pre}_{i}_bj"] = _rep(np.concatenate([np.asarray(q["b01"]),
                                                            np.asarray(q["b11"])]))
            weights[f"{pre}_{i}_w12"] = np.asarray(q["w12"], np.float32)
            weights[f"{pre}_{i}_b12"] = _rep(q["b12"])
    for name in ["conv1", "conv2", "conv3"]:
        weights[name] = _flat_w(np.asarray(p[name + "_w"], np.float32))
        weights[name + "_b"] = _rep(p[name + "_b"])
    for name in ["down0", "down1", "down2"]:
        weights[name] = _flat_w(np.asarray(p[name + "_w"], np.float32))
        weights[name + "_b"] = _rep(p[name + "_b"])
    db = {}
    for pre in [3, 4, 5, 6]:
        q = p[f"p{pre}"]
        weights[f"p{pre}_w1"] = np.asarray(q["w1"], np.float32)
        weights[f"p{pre}_b1"] = _rep(q["b1"])
        w2 = np.asarray(q["w2"], np.float32)
        b2 = np.asarray(q["b2"], np.float32)
        weights[f"p{pre}_dw"] = _rep(w2[:, 0] - w2[:, 1])
        db[pre] = float(b2[0] - b2[1])
        weights[f"p{pre}_db"] = np.full((P, 1), db[pre], np.float32)
    cfg["db"] = db
    cfg["weights"] = {k: v.shape for k, v in weights.items()}

    g0_full = x[np.asarray(inputs["nbr0_idx"])]
    maps = []
    for c in range(NC_CORES):
        m = {}
        m["g0"] = _pad_rows(g0_full[c * sh[0]:(c + 1) * sh[0]], tl[0], np.float32)
        m["offs_d0"] = _pad_rows(np.asarray(inputs["down0_idx"], np.int32)[c * sh[1]:(c + 1) * sh[1]], tl[1], np.int32)
        m["offs_l1"] = _pad_rows(np.asarray(inputs["nbr1_idx"], np.int32)[c * sh[1]:(c + 1) * sh[1]], tl[1], np.int32)
        m["offs_d1"] = _pad_rows(np.asarray(inputs["down1_idx"], np.int32)[c * sh[2]:(c + 1) * sh[2]], tl[2], np.int32)
        m["offs_l2"] = _pad_rows(np.asarray(inputs["nbr2_idx"], np.int32)[c * sh[2]:(c + 1) * sh[2]], tl[2], np.int32)
        m["offs_d2"] = _pad_rows(np.asarray(inputs["down2_idx"], np.int32)[c * sh[3]:(c + 1) * sh[3]], tl[3], np.int32)
        m["offs_l3"] = _pad_rows(np.asarray(inputs["nbr3_idx"], np.int32)[c * sh[3]:(c + 1) * sh[3]], tl[3], np.int32)
        m["noise_sh"] = _pad_rows(np.asarray(inputs["noise"], np.float32)[c * sh[3]:(c + 1) * sh[3]], tl[3], np.float32)
        for k, v in weights.items():
            m["w_" + k] = v
        maps.append(m)
    return maps


def kernel(x_feat, noise, params, nbr0_idx, down0_idx, nbr1_idx, down1_idx,
           nbr2_idx, down2_idx, nbr3_idx):
    inputs = dict(x_feat=x_feat, noise=noise, params=params, nbr0_idx=nbr0_idx,
                  down0_idx=down0_idx, nbr1_idx=nbr1_idx, down1_idx=down1_idx,
                  nbr2_idx=nbr2_idx, down2_idx=down2_idx, nbr3_idx=nbr3_idx)
    cfg = {"N": [int(np.asarray(nbr0_idx).shape[0]), int(np.asarray(nbr1_idx).shape[0]),
                 int(np.asarray(nbr2_idx).shape[0]), int(np.asarray(nbr3_idx).shape[0])]}
    maps = prepare(inputs, cfg)
    nc = build_net(cfg)
    res = run_bass_kernel_spmd(nc, maps, core_ids=list(range(NC_CORES)), trace=False)
    N = cfg["N"]
    sh = [n // NC_CORES for n in N]
    out0 = np.concatenate([np.asarray(res.results[c]["out0_sh"])[:sh[1]] for c in range(NC_CORES)])
    out1 = np.concatenate([np.asarray(res.results[c]["out1_sh"])[:sh[2]] for c in range(NC_CORES)])
    y = np.concatenate([np.asarray(res.results[c]["y_sh"])[:sh[3]] for c in range(NC_CORES)])
    comp = np.concatenate([np.asarray(res.results[c]["comp_sh"])[:sh[3]] for c in range(NC_CORES)])
    return (y, out1, out0, comp)
